# revision 1
# baseline (speedup 1.0000x reference)
"""ARIMA mse_loss kernel for 8 Trainium2 NeuronCores (nn_ARIMA_59373627900097).

Math (validated against the jax reference, rel err ~1e-7):
  For each t in [33, S): window v = y[t-32:t], target y[t].
    mean = sum(v)/32 ; var = sum(v^2)/32 - mean^2 ; std = sqrt(var + 1e-5)
    err_t = dotG_t - C1*std_t
  where dotG_t = sum_{j=0..32} G_j * y[t-32+j] is a single 33-tap FIR that
  folds the target (+1 tap), the AR filter (telescoped through the d=1
  differencing), and the RevIN mean removal.  The loss is
    ( sum(y[:33]^2) + sum_t err_t^2 ) / S.

Sharding: time axis split over 8 cores, 131040 predicted timesteps each
(96 outputs x 1365 matmul columns); the 223-step remainder plus the head
term are computed on host in float64.

Device layout per core: X[p, c] = y_slice[96*c + p] (fp16, host-prepared),
so each 128-row column holds 96 overlapping windows + targets.  Banded
(128x96) filter matrices turn the FIR + mean + E2 into TensorE matmuls.
The G filter is split hi/lo fp16 and accumulated in PSUM to keep the
systematic tap-quantization error ~1e-7.

Per chunk (four chunks of 512/512/256/85 columns, PSUM sets
double-buffered, engine programs software-pipelined two chunks deep):
  PE:      mean = Wm^T X      -> psA      (fp16 matmul, warmed up by dummy
           E2   = Wm^T X^2    -> psB       matmuls so HAM runs at 2.4GHz)
           dotG = Ghi^T X (+) Glo^T X -> psC  (PSUM accumulate)
  GpSimd:  X^2 (fp16)
  ScalarE: m2 = Square(psA);  std = Sqrt(var + eps);  sq_0 = Square(err)+accum
  DVE:     var = E2 - m2 (STT);  err = -C1*std + dotG (STT, C1 via AP);
           sq_1..3 = err*err with accum_out
  The per-partition partial sums of err^2 land in acc (96,4) -> DMA out.
"""

import numpy as np

P = 32
T0 = P + 1  # 33
S_TOTAL = 1048576
EPS_REVIN = 1e-5
EPS_W = 1e-10  # EPS*EPS in the reference denorm

N_CORES = 8
N_OUT = 96  # outputs per matmul column
N_COLS = 1365  # matmul columns per core
PER_CORE = N_OUT * N_COLS  # 131040 predicted timesteps per core
DATA_PER_CORE = 96 * (N_COLS - 1) + 128  # 131072 y values per core
CHUNKS = [(0, 512), (512, 512), (1024, 341)]  # (col start, ncols)

_CACHED = {}


def _taps(ar_weight, ar_bias, rev_weight, rev_bias):
    """33-tap err filter G, plus C1 (std coefficient), in float64."""
    aw = np.asarray(ar_weight, np.float64).reshape(-1)
    ab = float(np.asarray(ar_bias).reshape(-1)[0])
    w = float(np.asarray(rev_weight).reshape(-1)[0])
    b = float(np.asarray(rev_bias).reshape(-1)[0])
    c = np.zeros(P)
    c[0] = aw[0] - aw[1]
    for j in range(1, P - 1):
        c[j] = aw[j] - aw[j + 1]
    c[P - 1] = aw[P - 1]
    c[P - 2] += -1.0  # dser[:, -1] term
    c[P - 1] += +1.0
    F = c - aw[0] / P
    A = ab + b * aw[0]
    C1 = (A - b) / (w + EPS_W)
    C2 = w / (w + EPS_W)
    G = np.zeros(P + 1)
    G[:P] = -(C2 * F + 1.0 / P)
    G[P] = 1.0
    return G, C1


def _band(taps):
    """(128, 96) banded filter matrix: W[o + j, o] = taps[j]."""
    ntap = len(taps)
    W = np.zeros((128, N_OUT), np.float32)
    for o in range(N_OUT):
        W[o : o + ntap, o] = taps
    return W


def _weights(ar_weight, ar_bias, rev_weight, rev_bias):
    """(W fp16 (128,288), c1vec f32 (128,1), G f64, C1 float)."""
    G, C1 = _taps(ar_weight, ar_bias, rev_weight, rev_bias)
    G_hi32 = G.astype(np.float16).astype(np.float64)
    G_lo = (G - G_hi32).astype(np.float16).astype(np.float32)
    W = np.zeros((128, 288), np.float16)
    W[:, 0:96] = _band(np.full(P, 1.0 / P, np.float32)).astype(np.float16)
    W[:, 96:192] = _band(G_hi32.astype(np.float32)).astype(np.float16)
    W[:, 192:288] = _band(G_lo).astype(np.float16)
    c1vec = np.full((128, 1), -C1, np.float32)
    return W, c1vec, G, C1


def _shard_x(yf):
    """Per-core fp16 X tiles: X[p, c] = y16[1 + k*PER_CORE + 96 c + p]."""
    y16 = yf.astype(np.float16)
    xs = []
    for k in range(N_CORES):
        start = 1 + k * PER_CORE
        data = y16[start : start + DATA_PER_CORE]
        v = np.lib.stride_tricks.as_strided(
            data, shape=(N_COLS, 128), strides=(96 * 2, 2)
        )
        xs.append(np.ascontiguousarray(v.T))  # (128, N_COLS)
    return xs


def _build_program():
    import concourse.bass as bass
    from concourse import mybir

    f16 = mybir.dt.float16
    f32 = mybir.dt.float32
    Alu = mybir.AluOpType
    Act = mybir.ActivationFunctionType

    nc = bass.Bass("TRN2", target_bir_lowering=False, debug=False,
                   num_devices=N_CORES)

    xd = nc.dram_tensor("x", [128, N_COLS], f16, kind="ExternalInput")
    wd = nc.dram_tensor("w", [128, 288], f16, kind="ExternalInput")
    cd = nc.dram_tensor("c1", [128, 1], f32, kind="ExternalInput")
    od = nc.dram_tensor("out", [96, 4], f32, kind="ExternalOutput")

    xs = nc.alloc_sbuf_tensor("xs", [128, N_COLS], f16)
    x2 = nc.alloc_sbuf_tensor("x2", [128, N_COLS], f16)
    ws = nc.alloc_sbuf_tensor("ws", [128, 288], f16)
    c1s = nc.alloc_sbuf_tensor("c1s", [128, 1], f32)
    eps = nc.alloc_sbuf_tensor("eps", [128, 1], f32)
    acc = nc.alloc_sbuf_tensor("acc", [96, 4], f32)
    m2 = [nc.alloc_sbuf_tensor(f"m2_{s}", [96, 512], f32) for s in range(2)]
    var = [nc.alloc_sbuf_tensor(f"var_{s}", [96, 512], f32) for s in range(2)]
    std = [nc.alloc_sbuf_tensor(f"std_{s}", [96, 512], f32) for s in range(2)]
    scr = [nc.alloc_sbuf_tensor(f"scr_{s}", [96, 512], f32) for s in range(2)]

    psA = [nc.alloc_psum_tensor(f"psA{s}", [96, 512], f32) for s in range(2)]
    psB = [nc.alloc_psum_tensor(f"psB{s}", [96, 512], f32) for s in range(2)]
    psC = [nc.alloc_psum_tensor(f"psC{s}", [96, 512], f32) for s in range(2)]
    psD = nc.alloc_psum_tensor("psD", [96, 512], f32)  # warmup target

    warm = nc.alloc_sbuf_tensor("warm", [128, 512], f16)
    c0f = nc.const_aps.tensor(0.0, (128, 1), f32)

    # chunk layout: (col0, ncols, psum set); the shrinking tail keeps the
    # final dependency chain short
    CH = [(0, 512, 0), (512, 512, 1), (1024, 256, 0), (1280, 85, 1)]

    with (
        nc.Block() as block,
        nc.semaphore("s_dw") as s_dw,
        nc.semaphore("s_dc") as s_dc,
        nc.semaphore("s_dx0") as s_dx0,
        nc.semaphore("s_dx1") as s_dx1,
        nc.semaphore("s_dx2") as s_dx2,
        nc.semaphore("s_do") as s_do,
        nc.semaphore("s_x2") as s_x2,
        nc.semaphore("s_pe") as s_pe,
        nc.semaphore("s_sc") as s_sc,
        nc.semaphore("s_v") as s_v,
    ):
        s_dx = [s_dx0, s_dx1, s_dx2, s_dx2]

        @block.sync
        def _(sync):
            sync.dma_start(
                out=xs.ap()[:, 0:512], in_=xd.ap()[:, 0:512]
            ).then_inc(s_dx0, 16)
            sync.dma_start(
                out=xs.ap()[:, 512:1024], in_=xd.ap()[:, 512:1024]
            ).then_inc(s_dx1, 16)
            sync.dma_start(out=c1s.ap(), in_=cd.ap()).then_inc(s_dc, 16)
            # final output DMA after all accumulators are written
            sync.wait_ge(s_sc, 5)
            sync.wait_ge(s_v, 11)
            sync.dma_start(out=od.ap(), in_=acc.ap()).then_inc(s_do, 16)
            sync.wait_ge(s_do, 16)

        @block.gpsimd
        def _(g):
            g.memset(warm.ap(), 0.0).then_inc(s_x2, 1)
            g.memset(eps.ap(), EPS_REVIN).then_inc(s_x2, 1)
            for ci, (c0, fc) in enumerate([(0, 512), (512, 512), (1024, 341)]):
                g.wait_ge(s_dx[ci], 16)
                g.tensor_tensor(
                    x2.ap()[:, c0 : c0 + fc],
                    xs.ap()[:, c0 : c0 + fc],
                    xs.ap()[:, c0 : c0 + fc],
                    Alu.mult,
                ).then_inc(s_x2, 1)

        @block.tensor
        def _(t):
            # PE warmup during the DMA wait: full 512-col matmuls on a zero
            # tile keep the HAM activity window busy so the real matmuls run
            # at 2.4GHz instead of 1.2.
            t.wait_ge(s_x2, 1)
            for _ in range(4):
                t.matmul(psD.ap(), warm.ap()[:, 0:96], warm.ap(),
                         start=True, stop=True)
            t.wait_ge(s_dw, 16)  # weights
            # psum-set reuse waits: chunk2 needs chunk0's consumers done,
            # chunk3 needs chunk1's
            reuse = {2: (3, 1), 3: (4, 2)}
            x2w = {0: 3, 1: 4, 2: 5, 3: 5}
            for ci, (c0, fc, s) in enumerate(CH):
                xsl = xs.ap()[:, c0 : c0 + fc]
                if ci in reuse:
                    rv, rs = reuse[ci]
                    t.wait_ge(s_v, rv)
                    t.wait_ge(s_sc, rs)
                t.wait_ge(s_dx[ci], 16)  # x chunk
                t.matmul(psA[s].ap()[:, :fc], ws.ap()[:, 0:96], xsl,
                         start=True, stop=True).then_inc(s_pe, 1)
                t.matmul(psC[s].ap()[:, :fc], ws.ap()[:, 96:192], xsl,
                         start=True, stop=False).then_inc(s_pe, 1)
                t.matmul(psC[s].ap()[:, :fc], ws.ap()[:, 192:288], xsl,
                         start=False, stop=True).then_inc(s_pe, 1)
                t.wait_ge(s_x2, x2w[ci])
                t.matmul(psB[s].ap()[:, :fc], ws.ap()[:, 0:96],
                         x2.ap()[:, c0 : c0 + fc],
                         start=True, stop=True).then_inc(s_pe, 1)

        @block.scalar
        def _(sc):
            sc.dma_start(out=ws.ap(), in_=wd.ap()).then_inc(s_dw, 16)
            sc.dma_start(
                out=xs.ap()[:, 1024:1365], in_=xd.ap()[:, 1024:1365]
            ).then_inc(s_dx2, 16)
            # dummy activation: pulls the ACT table load off the critical path
            sc.activation(scr[0].ap()[:, 0:1], c0f[:96], Act.Square)
            # software-pipelined schedule: both m2 of a chunk pair run
            # back-to-back, then both sqrts, so ScalarE and VectorE ping-pong
            # between chunk pairs instead of stalling inside one chain
            F = [512, 512, 256, 85]
            sc.wait_ge(s_pe, 1)
            sc.activation(m2[0].ap()[:, :F[0]], psA[0].ap()[:, :F[0]],
                          Act.Square).then_inc(s_sc, 1)        # sc1 m2_0
            sc.wait_ge(s_pe, 5)
            sc.activation(m2[1].ap()[:, :F[1]], psA[1].ap()[:, :F[1]],
                          Act.Square).then_inc(s_sc, 1)        # sc2 m2_1
            sc.wait_ge(s_v, 1)
            sc.activation(std[0].ap()[:, :F[0]], var[0].ap()[:, :F[0]],
                          Act.Sqrt, bias=eps.ap()[:96]).then_inc(s_sc, 1)  # sc3 sqrt_0
            sc.wait_ge(s_v, 2)
            sc.activation(std[1].ap()[:, :F[1]], var[1].ap()[:, :F[1]],
                          Act.Sqrt, bias=eps.ap()[:96]).then_inc(s_sc, 1)  # sc4 sqrt_1
            sc.wait_ge(s_v, 3)
            sc.activation(scr[0].ap()[:, :F[0]], var[0].ap()[:, :F[0]],
                          Act.Square,
                          accum_out=acc.ap()[:, 0:1]).then_inc(s_sc, 1)  # sc5 sq_0
            sc.wait_ge(s_pe, 9)
            sc.activation(m2[0].ap()[:, :F[2]], psA[0].ap()[:, :F[2]],
                          Act.Square).then_inc(s_sc, 1)        # sc6 m2_2
            sc.wait_ge(s_pe, 13)
            sc.activation(m2[1].ap()[:, :F[3]], psA[1].ap()[:, :F[3]],
                          Act.Square).then_inc(s_sc, 1)        # sc7 m2_3
            sc.wait_ge(s_v, 6)
            sc.activation(std[0].ap()[:, :F[2]], var[0].ap()[:, :F[2]],
                          Act.Sqrt, bias=eps.ap()[:96]).then_inc(s_sc, 1)  # sc8 sqrt_2
            sc.wait_ge(s_v, 7)
            sc.activation(std[1].ap()[:, :F[3]], var[1].ap()[:, :F[3]],
                          Act.Sqrt, bias=eps.ap()[:96]).then_inc(s_sc, 1)  # sc9 sqrt_3

        @block.vector
        def _(v):
            v.wait_ge(s_dc, 16)
            F = [512, 512, 256, 85]

            def var_op(ci, s, pe, scw):
                v.wait_ge(s_pe, pe)
                v.wait_ge(s_sc, scw)
                v.scalar_tensor_tensor(
                    var[s].ap()[:, :F[ci]], m2[s].ap()[:, :F[ci]], -1.0,
                    psB[s].ap()[:, :F[ci]], Alu.mult, Alu.add,
                ).then_inc(s_v, 1)

            def err_op(ci, s, scw):
                v.wait_ge(s_sc, scw)
                v.scalar_tensor_tensor(
                    var[s].ap()[:, :F[ci]], std[s].ap()[:, :F[ci]],
                    c1s.ap()[:96], psC[s].ap()[:, :F[ci]], Alu.mult, Alu.add,
                ).then_inc(s_v, 1)

            def sq_op(ci, s, vw):
                v.wait_ge(s_v, vw)
                v.scalar_tensor_tensor(
                    scr[s].ap()[:, :F[ci]], var[s].ap()[:, :F[ci]], 1.0,
                    var[s].ap()[:, :F[ci]], Alu.bypass, Alu.mult,
                    accum_out=acc.ap()[:, ci : ci + 1],
                ).then_inc(s_v, 1)

            var_op(0, 0, 4, 1)    # v1
            var_op(1, 1, 8, 2)    # v2
            err_op(0, 0, 3)       # v3
            err_op(1, 1, 4)       # v4
            sq_op(1, 1, 4)        # v5
            var_op(2, 0, 12, 6)   # v6
            v.wait_ge(s_v, 5)     # order after sq_1 (reads var[1])
            var_op(3, 1, 16, 7)   # v7
            err_op(2, 0, 8)       # v8
            err_op(3, 1, 9)       # v9
            sq_op(2, 0, 8)        # v10
            sq_op(3, 1, 9)        # v11

    return nc


def kernel(y, ar_weight, ar_bias, rev_weight, rev_bias):
    yf = np.asarray(y, np.float32).reshape(-1)
    S = yf.shape[0]
    assert S == S_TOTAL, f"kernel hardcoded for S={S_TOTAL}, got {S}"

    W, c1vec, G, C1 = _weights(ar_weight, ar_bias, rev_weight, rev_bias)
    xsh = _shard_x(yf)
    in_maps = [{"x": xsh[k], "w": W, "c1": c1vec} for k in range(N_CORES)]

    if "nc" not in _CACHED:
        _CACHED["nc"] = _build_program()
    nc = _CACHED["nc"]

    import os

    # keep run_bass_kernel_spmd on the plain (non-NTFF-trace) path; the
    # trace path needs hooks this container may not have installed
    os.environ["BASS_NEVER_TRACE"] = "1"
    from concourse.bass_utils import run_bass_kernel_spmd

    try:
        res = run_bass_kernel_spmd(nc, in_maps, list(range(N_CORES)))
    except Exception:
        # transient device/terminal hiccups happen; one retry
        import time

        time.sleep(5)
        res = run_bass_kernel_spmd(nc, in_maps, list(range(N_CORES)))

    total = 0.0
    for k in range(N_CORES):
        total += float(res.results[k]["out"].astype(np.float64).sum())

    y64 = yf.astype(np.float64)
    head = float((y64[:T0] ** 2).sum())

    # host tail: t in [33 + 8*PER_CORE, S)
    t_start = T0 + N_CORES * PER_CORE
    n_tail = S - t_start
    if n_tail > 0:
        idx = (t_start - P) + np.arange(n_tail)[:, None] + np.arange(P)[None, :]
        win = y64[idx]
        mean = win.mean(axis=1)
        var = win.var(axis=1)
        stdv = np.sqrt(var + EPS_REVIN)
        idx33 = (t_start - P) + np.arange(n_tail)[:, None] + np.arange(P + 1)[None, :]
        dotG = y64[idx33] @ G
        err = dotG - C1 * stdv
        total += float((err**2).sum())

    loss = (head + total) / S
    return np.array(loss, dtype=np.float32)

